# revision 13
# baseline (speedup 1.0000x reference)
"""MLA encoder self-attention on 8 TRN2 NeuronCores.

Sharding: data-parallel over batch (B=2) x tensor-parallel over head groups
(16 heads -> 4 groups of 4). Core c handles batch c//4, heads 4*(c%4)..+4.
Each core computes its heads' attention and a partial output projection;
the host sums the 4 head-group partials per batch.

v3 (vs v1 baseline):
- All DRAM traffic and matmul operands are bf16 (halves DMA, enables PE
  fast-weight-load); accumulation stays fp32 in PSUM.
- Activation engine runs only LN/Sqrt/Exp (Exp is its serial floor in
  the attention phase); PSUM evacuations split between ACT (stage 1/2)
  and DVE (attention phase).
- DMA queues: x on sync, stage-1 weights on scalar/gpsimd FIRST, then
  stage-2 weights + Aq/Bq rope tables prefetched behind them.
- The q decompress+rope path runs in token-tile pairs; its matmuls and
  transposes are separate fill units interleaved (with out-projection
  chunks) into the attention u-loops, so PE fills the exp-wait gaps.
  All fills share one [128,512]-f32 PSUM scratch tag (2 banks), fitting
  the 8-bank budget with double-buffered scores (4) + AV accumulators
  (2).
- Output is written bf16; host accumulates partials in fp32.

Key performance structure (unchanged from v1):
- Attention scores for the two heads of a pair use K=64 contractions at
  PE row-groups (0,*) and (64,*) issued back-to-back.
- The two heads' score tiles land in one [128,1024] PSUM tile, so one
  Exp activation covers both; softmax denominator comes from an
  appended ones-column in kvd (row 64 of the AV accumulation),
  normalization deferred to right before the output projection.
"""
import numpy as np

B, S, E = 2, 2048, 1024
H, HD = 16, 64
ROPE, NOPE = 32, 32
QL, KVL = 512, 512
EPS = 1e-5
HPC = 4                # heads per core
DPC = HPC * HD         # 256
NCORES = 8
TT = S // 128          # 16 token tiles
ET = E // 128          # 8
RT = QL // 128         # 4
SB = S // 512          # 4 s-blocks
UT = S // 128          # 16 key tiles
KVW = 4 * 97           # 388: kvd width

_CACHE = {}


def _build(reps=1):
    import concourse.tile as tile
    import concourse.mybir as mybir
    from concourse import bacc

    f32 = mybir.dt.float32
    bf16 = mybir.dt.bfloat16
    AF = mybir.ActivationFunctionType
    ALU = mybir.AluOpType

    nc = bacc.Bacc("TRN2", target_bir_lowering=False, debug=False,
                   num_devices=NCORES)

    def din(name, shape, dt=bf16):
        return nc.dram_tensor(name, shape, dt, kind="ExternalInput").ap()

    xT_d = din("xT", (E, S))
    wqa_d = din("WqaT", (E, QL))
    wkva_d = din("WkvaT", (E, KVL + ROPE))
    wqb_d = din("WqbT", (QL, DPC))
    qconst_d = din("qconst", (1, DPC))
    wkvb_d = din("WkvbT", (KVL, KVW))
    kconst_d = din("kconst", (1, KVW))
    wout_d = din("WoutT", (DPC, E))
    aq_d = din("Aq", (S, DPC))
    bq_d = din("Bq", (S, DPC))
    c2k_d = din("c2k", (S, ROPE))
    s2k_d = din("s2k", (S, ROPE))
    ident_d = din("ident", (128, 128))
    ones1_d = din("ones1", (1, 128))
    eps_d = din("epst", (128, 1), mybir.dt.float32)
    out_d = nc.dram_tensor("out", (S, E), bf16, kind="ExternalOutput").ap()

    with tile.TileContext(nc) as tc:
        import contextlib
        with contextlib.ExitStack() as top:
            consts = top.enter_context(tc.tile_pool(name="consts", bufs=1))
            ident_b = consts.tile([128, 128], bf16, tag="ident_b")
            ones1_b = consts.tile([1, 128], bf16, tag="ones1")
            nc.vector.memset(ones1_b[:], 1.0)
            qconst_b = consts.tile([1, DPC], bf16, tag="qconst")
            kconst_b = consts.tile([1, KVW], bf16, tag="kconst")
            eps_t = consts.tile([128, 1], f32, tag="epst")
            nc.vector.memset(eps_t[:], EPS)

            acts = top.enter_context(tc.tile_pool(name="acts", bufs=1))
            wq2 = top.enter_context(tc.tile_pool(name="wq2", bufs=1))
            wqa_p = top.enter_context(tc.tile_pool(name="wqa", bufs=1))
            ck = top.enter_context(tc.tile_pool(name="ck", bufs=1))
            qp = top.enter_context(tc.tile_pool(name="qp", bufs=2))

            def body():
                # feature-major stage-1 outputs: [128, RT, S] (r on mid dim)
                qcnT = acts.tile([128, RT, S], bf16, tag="qcnT", name="qcnT")
                ckvnT = acts.tile([128, RT, S], bf16, tag="ckvnT",
                                  name="ckvnT")
                kpe = [acts.tile([128, ROPE], bf16, tag=f"kpe{t}",
                                 name=f"kpe{t}")
                       for t in range(TT)]

                # stage-1 weights first on the scalar/gpsimd queues (the
                # first matmuls wait on e-slice 0)
                wqa_t = wqa_p.tile([128, ET, QL], bf16, tag="wqa")
                wkva_t = wqa_p.tile([128, ET, KVL + ROPE], bf16, tag="wkva")
                for e in range(ET):
                    es_ = slice(e * 128, (e + 1) * 128)
                    nc.scalar.dma_start(wqa_t[:, e, :], wqa_d[es_, :])
                    nc.gpsimd.dma_start(wkva_t[:, e, :], wkva_d[es_, :])
                nc.scalar.dma_start(ident_b[:], ident_d[:])
                nc.scalar.dma_start(qconst_b[:], qconst_d[:])
                nc.scalar.dma_start(kconst_b[:], kconst_d[:])
                c2k_t = ck.tile([128, TT, ROPE], bf16, tag="c2k")
                nc.scalar.dma_start(
                    c2k_t[:], c2k_d.rearrange("(t p) r -> p t r", p=128))
                s2k_t = ck.tile([128, TT, ROPE], bf16, tag="s2k")
                nc.scalar.dma_start(
                    s2k_t[:], s2k_d.rearrange("(t p) r -> p t r", p=128))

                # stage-2 weights + rope tables prefetch behind them; they
                # land while stage 1 computes.
                wqb_b = wq2.tile([128, RT, DPC], bf16, tag="wqb")
                nc.scalar.dma_start(
                    wqb_b[:], wqb_d.rearrange("(k p) n -> p k n", p=128))
                wkvb_b = wq2.tile([128, RT, KVW], bf16, tag="wkvb")
                nc.scalar.dma_start(
                    wkvb_b[:], wkvb_d.rearrange("(k p) n -> p k n", p=128))
                wout_b = wq2.tile([128, 2, E], bf16, tag="wout")
                nc.scalar.dma_start(
                    wout_b[:], wout_d.rearrange("(k p) n -> p k n", p=128))
                aq_t = wq2.tile([128, TT, DPC], bf16, tag="aq")
                nc.gpsimd.dma_start(
                    aq_t[:], aq_d.rearrange("(t p) d -> p t d", p=128))
                bq_t = wq2.tile([128, TT, DPC], bf16, tag="bq")
                nc.gpsimd.dma_start(
                    bq_t[:], bq_d.rearrange("(t p) d -> p t d", p=128))

                # ---------------- stage 1: qa + kva + LN + kpe rope -------
                with contextlib.ExitStack() as st1:
                    pmm = st1.enter_context(
                        tc.tile_pool(name="pmm", bufs=2, space="PSUM"))
                    ptp = st1.enter_context(
                        tc.tile_pool(name="ptp", bufs=1, space="PSUM"))
                    xp = st1.enter_context(tc.tile_pool(name="xp", bufs=3))
                    sp = st1.enter_context(tc.tile_pool(name="sp", bufs=3))

                    pend = []

                    def do_tp1(t, qcn_t, ckvn_t):
                        ts_ = slice(t * 128, (t + 1) * 128)
                        tpq = ptp.tile([128, QL], bf16, tag="tpq",
                                       name=f"tpq_{t}")
                        tpk = ptp.tile([128, QL], bf16, tag="tpk",
                                       name=f"tpk_{t}")
                        for r in range(RT):
                            rs = slice(r * 128, (r + 1) * 128)
                            nc.tensor.transpose(tpq[:, rs], qcn_t[:, rs],
                                                ident_b[:])
                            nc.tensor.transpose(tpk[:, rs], ckvn_t[:, rs],
                                                ident_b[:])
                        nc.scalar.activation(
                            qcnT[:, :, ts_],
                            tpq.rearrange("p (r c) -> p r c", r=RT)[:],
                            AF.Copy)
                        nc.vector.tensor_copy(
                            ckvnT[:, :, ts_],
                            tpk.rearrange("p (r c) -> p r c", r=RT)[:])

                    for t in range(TT):
                        ts_ = slice(t * 128, (t + 1) * 128)
                        p_qc = pmm.tile([128, QL], f32, tag="p_qc")
                        p_ka = pmm.tile([128, 272], f32, tag="p_ka")
                        p_kb = pmm.tile([128, 272], f32, tag="p_kb")
                        xt = xp.tile([128, ET, 128], bf16, tag="xt")
                        if t == 0:
                            for e in range(ET):
                                es_ = slice(e * 128, (e + 1) * 128)
                                nc.sync.dma_start(
                                    xt[:, e, :], xT_d[es_, ts_])
                        else:
                            nc.sync.dma_start(
                                xt[:],
                                xT_d.rearrange("(k p) s -> p k s",
                                               p=128)[:, :, ts_])
                        for e in range(ET):
                            st, sp_ = (e == 0), (e == ET - 1)
                            nc.tensor.matmul(p_qc[:], xt[:, e, :],
                                             wqa_t[:, e, :],
                                             start=st, stop=sp_)
                            nc.tensor.matmul(p_ka[:], xt[:, e, :],
                                             wkva_t[:, e, 0:272],
                                             start=st, stop=sp_)
                            nc.tensor.matmul(p_kb[:], xt[:, e, :],
                                             wkva_t[:, e, 272:544],
                                             start=st, stop=sp_)
                        # LN on qc
                        stq = sp.tile([128, 6], f32, tag="stq")
                        nc.vector.bn_stats(stq[:], p_qc[:])
                        mvq = sp.tile([128, 2], f32, tag="mvq")
                        nc.vector.bn_aggr(mvq[:], stq[:])
                        rsq = sp.tile([128, 1], f32, tag="rsq")
                        nc.scalar.activation(rsq[:], mvq[:, 1:2], AF.Sqrt,
                                             bias=eps_t[:])
                        rsq2 = sp.tile([128, 1], f32, tag="rsq2")
                        nc.vector.reciprocal(rsq2[:], rsq[:])
                        nmq = sp.tile([128, 1], f32, tag="nmq")
                        nc.vector.tensor_scalar(
                            out=nmq[:], in0=mvq[:, 0:1], scalar1=rsq2[:],
                            scalar2=-1.0, op0=ALU.mult, op1=ALU.mult)
                        qcn_t = sp.tile([128, QL], bf16, tag="qcn", bufs=3)
                        nc.scalar.activation(qcn_t[:], p_qc[:], AF.Identity,
                                             bias=nmq[:], scale=rsq2[:])
                        # LN on ckv (272 + 240 chunks)
                        stk = sp.tile([128, 2, 6], f32, tag="stk")
                        nc.vector.bn_stats(stk[:, 0, :], p_ka[:])
                        nc.vector.bn_stats(stk[:, 1, :], p_kb[:, 0:240])
                        mvk = sp.tile([128, 2], f32, tag="mvk")
                        nc.vector.bn_aggr(mvk[:], stk[:])
                        rsk = sp.tile([128, 1], f32, tag="rsk")
                        nc.scalar.activation(rsk[:], mvk[:, 1:2], AF.Sqrt,
                                             bias=eps_t[:])
                        rsk2 = sp.tile([128, 1], f32, tag="rsk2")
                        nc.vector.reciprocal(rsk2[:], rsk[:])
                        nmk = sp.tile([128, 1], f32, tag="nmk")
                        nc.vector.tensor_scalar(
                            out=nmk[:], in0=mvk[:, 0:1], scalar1=rsk2[:],
                            scalar2=-1.0, op0=ALU.mult, op1=ALU.mult)
                        ckvn_t = sp.tile([128, KVL], bf16, tag="ckvn", bufs=3)
                        nc.scalar.activation(ckvn_t[:, 0:272], p_ka[:],
                                             AF.Identity, bias=nmk[:],
                                             scale=rsk2[:])
                        nc.scalar.activation(ckvn_t[:, 272:512], p_kb[:, 0:240],
                                             AF.Identity, bias=nmk[:],
                                             scale=rsk2[:])
                        # kpe rope (raw cols 240:272 of p_kb)
                        kraw = sp.tile([128, ROPE], bf16, tag="kraw")
                        nc.vector.tensor_copy(kraw[:], p_kb[:, 240:272])
                        ksw = sp.tile([128, ROPE], bf16, tag="ksw")
                        kraw3 = kraw.rearrange("p (i two) -> p i two", two=2)
                        ksw3 = ksw.rearrange("p (i two) -> p i two", two=2)
                        nc.gpsimd.tensor_copy(ksw3[:, :, 0:1], kraw3[:, :, 1:2])
                        nc.gpsimd.tensor_copy(ksw3[:, :, 1:2], kraw3[:, :, 0:1])
                        c2t = c2k_t[:, t, :]
                        s2t = s2k_t[:, t, :]
                        kp1 = sp.tile([128, ROPE], bf16, tag="kp1")
                        nc.gpsimd.tensor_mul(kp1[:], kraw[:], c2t[:])
                        kp2 = sp.tile([128, ROPE], bf16, tag="kp2")
                        nc.gpsimd.tensor_mul(kp2[:], ksw[:], s2t[:])
                        nc.gpsimd.tensor_add(kpe[t][:], kp1[:], kp2[:])
                        pend.append((t, qcn_t, ckvn_t))
                        if len(pend) > 2:
                            do_tp1(*pend.pop(0))
                    for args in pend:
                        do_tp1(*args)

                # ---------------- q path (pairs of token tiles) -----------
                acts2 = top.enter_context(tc.tile_pool(name="acts2", bufs=1))
                qfT = acts2.tile([128, 2, S], bf16, tag="qfT", name="qfT")
                kfT = acts2.tile([128, 2, S], bf16, tag="kfT", name="kfT")
                kvd = [acts2.tile([128, KVW], bf16, tag=f"kvd{t}",
                                  name=f"kvd{t}")
                       for t in range(TT)]

                def q_pair_mm(tp, pool, tag="p_s2"):
                    """q decompress + rope for token tiles 2tp, 2tp+1.
                    Returns qf_t for the later transpose fill unit."""
                    t0 = 2 * tp
                    p_q = pool.tile([128, 512], f32, tag=tag,
                                    name=f"p_q_{tp}")
                    for half in range(2):
                        hs = slice(half * DPC, (half + 1) * DPC)
                        hts = slice((t0 + half) * 128, (t0 + half + 1) * 128)
                        for r in range(RT):
                            nc.tensor.matmul(p_q[:, hs], qcnT[:, r, hts],
                                             wqb_b[:, r, :],
                                             start=(r == 0), stop=False)
                        nc.tensor.matmul(p_q[:, hs], ones1_b[:], qconst_b[:],
                                         start=False, stop=True)
                    q_t = qp.tile([128, 512], bf16, tag="q_t")
                    nc.vector.tensor_copy(q_t[:], p_q[:])
                    # rope (Aq/Bq carry cos/+-sin on rope cols, 1/0 on nope)
                    a_t = aq_t[:, t0:t0 + 2, :].rearrange("p t d -> p (t d)")
                    b_t = bq_t[:, t0:t0 + 2, :].rearrange("p t d -> p (t d)")
                    q_sw = qp.tile([128, 512], bf16, tag="q_sw")
                    q3 = q_t.rearrange("p (i two) -> p i two", two=2)
                    qs3 = q_sw.rearrange("p (i two) -> p i two", two=2)
                    nc.gpsimd.tensor_copy(qs3[:, :, 0:1], q3[:, :, 1:2])
                    nc.gpsimd.tensor_copy(qs3[:, :, 1:2], q3[:, :, 0:1])
                    qt1 = qp.tile([128, 512], bf16, tag="qt1")
                    nc.vector.tensor_mul(qt1[:], q_t[:], a_t)
                    qt2 = qp.tile([128, 512], bf16, tag="qt2")
                    nc.vector.tensor_mul(qt2[:], q_sw[:], b_t)
                    qf_t = qp.tile([128, 512], bf16, tag="qf_t", bufs=3)
                    nc.vector.tensor_add(qf_t[:], qt1[:], qt2[:])
                    return qf_t

                def q_pair_tp(tp, pool, qf_t, tag="p_s2"):
                    """Transpose fill unit: qf pair -> qfT (emitted a few
                    u-steps after q_pair_mm so rope has completed)."""
                    t0 = 2 * tp
                    ss = slice(t0 * 128, (t0 + 2) * 128)
                    tpq = pool.tile([128, 512], bf16, tag=tag,
                                    name=f"tpq2_{tp}")
                    for c in range(4):
                        cs = slice(c * 128, (c + 1) * 128)
                        nc.tensor.transpose(tpq[:, cs], qf_t[:, cs],
                                            ident_b[:])
                    # tpq cols: (t0 j0, t0 j1, t1 j0, t1 j1)
                    nc.vector.tensor_copy(
                        qfT[:, :, ss].rearrange("p j (t c) -> p j t c", t=2),
                        tpq.rearrange("p (t j c) -> p j t c", t=2, j=2)[:])

                # ------- stage 2 (kvb + k assembly) merged with the ----
                # ------- first attention u-loop (sb0, j0): exp starts ----
                # ------- while kv tiles are still decompressing ----------
                with contextlib.ExitStack() as st2:
                    pkv = st2.enter_context(
                        tc.tile_pool(name="pkv", bufs=1, space="PSUM"))
                    ptp2 = st2.enter_context(
                        tc.tile_pool(name="ptp2", bufs=1, space="PSUM"))
                    ps_s = st2.enter_context(
                        tc.tile_pool(name="ps_s", bufs=2, space="PSUM"))
                    ps_av = st2.enter_context(
                        tc.tile_pool(name="ps_av", bufs=1, space="PSUM"))
                    qk2 = st2.enter_context(tc.tile_pool(name="qk2", bufs=2))
                    ex = st2.enter_context(tc.tile_pool(name="ex", bufs=3))
                    on = st2.enter_context(tc.tile_pool(name="on", bufs=4))
                    ozs = st2.enter_context(tc.tile_pool(name="ozs", bufs=4))
                    osb = st2.enter_context(tc.tile_pool(name="osb", bufs=2))

                    pend2 = []

                    def do_tpk(t, kf_t):
                        ts_ = slice(t * 128, (t + 1) * 128)
                        tpk = ptp2.tile([128, DPC], bf16, tag="tp2k",
                                        name=f"tpk2_{t}")
                        for j in range(2):
                            js = slice(j * 128, (j + 1) * 128)
                            nc.tensor.transpose(tpk[:, js], kf_t[:, js],
                                                ident_b[:])
                        nc.vector.tensor_copy(
                            kfT[:, :, ts_],
                            tpk.rearrange("p (j c) -> p j c", j=2)[:])

                    def kv_tile(t):
                        ts_ = slice(t * 128, (t + 1) * 128)
                        p_kv = pkv.tile([128, KVW], f32, tag="p_kv")
                        for r in range(RT):
                            nc.tensor.matmul(p_kv[:], ckvnT[:, r, ts_],
                                             wkvb_b[:, r, :],
                                             start=(r == 0), stop=False)
                        # kconst has 1.0 at the ones positions (col 96 of
                        # each head) so kvd's softmax-denominator column
                        # needs no separate write.
                        nc.tensor.matmul(p_kv[:], ones1_b[:], kconst_b[:],
                                         start=False, stop=True)
                        nc.scalar.activation(kvd[t][:], p_kv[:], AF.Copy)
                        # k_full assembly: nope part from kvd (SBUF, Pool),
                        # rope part broadcast from kpe.
                        kf_t = qk2.tile([128, DPC], bf16, tag="kf_t", bufs=3)
                        kf3 = kf_t.rearrange("p (h c) -> p h c", h=HPC)
                        kvd3 = kvd[t].rearrange("p (h c) -> p h c", h=HPC)
                        nc.gpsimd.tensor_copy(kf3[:, :, 0:32],
                                              kvd3[:, :, 0:32])
                        for h in range(HPC):
                            nc.gpsimd.tensor_copy(
                                kf_t[:, h * 64 + 32:h * 64 + 64], kpe[t][:])
                        pend2.append((t, kf_t))
                        if len(pend2) > 2:
                            do_tpk(*pend2.pop(0))

                    def attn_u(sb_i, j, u, p_avA, p_avB, ss):
                        hA, hB = 2 * j, 2 * j + 1
                        us = slice(u * 128, (u + 1) * 128)
                        p_s2 = ps_s.tile([128, 1024], f32, tag="p_s2")
                        # two heads' scores: K=64 row-groups (0,*) and
                        # (64,*) run concurrently on the PE
                        nc.tensor.matmul(p_s2[:, 0:512],
                                         kfT[0:64, j, us],
                                         qfT[0:64, j, ss],
                                         start=True, stop=True)
                        nc.tensor.matmul(p_s2[:, 512:1024],
                                         kfT[64:128, j, us],
                                         qfT[64:128, j, ss],
                                         start=True, stop=True)
                        e2 = ex.tile([128, 1024], bf16, tag="e2")
                        nc.scalar.activation(e2[:], p_s2[:], AF.Exp,
                                             scale=0.125)
                        return e2

                    def attn_av(u, e2, p_avA, p_avB, hA, hB):
                        nc.tensor.matmul(
                            p_avA[0:65, :],
                            kvd[u][:, hA * 97 + 32:hA * 97 + 97],
                            e2[:, 0:512],
                            start=(u == 0), stop=(u == UT - 1))
                        nc.tensor.matmul(
                            p_avB[0:65, :],
                            kvd[u][:, hB * 97 + 32:hB * 97 + 97],
                            e2[:, 512:1024],
                            start=(u == 0), stop=(u == UT - 1))

                    def norms(j, p_avA, p_avB, onorm):
                        # evacuate raw AV (+denominator row) first so the
                        # PSUM banks free for the next j's accumulation
                        for half, p_av in ((0, p_avA), (64, p_avB)):
                            hs = slice(half, half + 64)
                            avr = ozs.tile([65, 512], f32, tag="avr",
                                           bufs=2)
                            nc.vector.tensor_copy(avr[:], p_av[0:65, :])
                            rz = ozs.tile([1, 512], f32, tag="rz")
                            with nc.allow_low_precision(reason="bf16 Z"):
                                nc.vector.reciprocal(rz[:], avr[64:65, :])
                            zb = ozs.tile([64, 512], f32, tag="zb")
                            nc.gpsimd.partition_broadcast(zb[:], rz[:],
                                                          channels=64)
                            nc.vector.tensor_mul(onorm[j][hs, :],
                                                 avr[0:64, :], zb[:])

                    def outproj_chunk(pool, sb_i, onorm, tc_i, ei, tag):
                        es = slice(ei * 512, (ei + 1) * 512)
                        tcs = slice(tc_i * 128, (tc_i + 1) * 128)
                        p_o = pool.tile([128, 512], f32, tag=tag,
                                        name=f"p_o_{sb_i}_{tc_i}_{ei}")
                        for kk in range(2):
                            nc.tensor.matmul(
                                p_o[:], onorm[kk][:, tcs],
                                wout_b[:, kk, es],
                                start=(kk == 0), stop=(kk == 1))
                        o_t = osb.tile([128, 512], bf16, tag="o_t",
                                       name=f"o_t_{sb_i}_{tc_i}_{ei}")
                        nc.vector.tensor_copy(o_t[:], p_o[:])
                        nc.sync.dma_start(
                            out_d[sb_i * 512 + tc_i * 128:
                                  sb_i * 512 + tc_i * 128 + 128, es],
                            o_t[:])

                    # merged loop: kv tiles + q pairs 0/1 + (sb0, j0)
                    # attention steps u = t-4
                    onorm0 = [on.tile([128, 512], bf16, tag=f"on{j}",
                                      name=f"on{j}_0")
                              for j in range(2)]
                    ss0 = slice(0, 512)
                    av00A = ps_av.tile([128, 512], f32, tag="p_avA",
                                       name="p_avA_0_0")
                    av00B = ps_av.tile([128, 512], f32, tag="p_avB",
                                       name="p_avB_0_0")
                    qf01 = {}
                    for t in range(TT):
                        kv_tile(t)
                        if t == 0:
                            qf01[0] = q_pair_mm(0, ps_s)
                        elif t == 1:
                            q_pair_tp(0, ps_s, qf01[0])
                            qf01[1] = q_pair_mm(1, ps_s)
                        elif t == 2:
                            q_pair_tp(1, ps_s, qf01[1])
                        if t >= 4:
                            u = t - 4
                            e2 = attn_u(0, 0, u, av00A, av00B, ss0)
                            attn_av(u, e2, av00A, av00B, 0, 1)
                    for args in pend2:
                        do_tpk(*args)
                    for u in range(TT - 4, UT):
                        e2 = attn_u(0, 0, u, av00A, av00B, ss0)
                        attn_av(u, e2, av00A, av00B, 0, 1)
                    norms(0, av00A, av00B, onorm0)

                # ---------------- remaining attention + fills -------------
                with contextlib.ExitStack() as st3:
                    ps_s = st3.enter_context(
                        tc.tile_pool(name="ps_s3", bufs=2, space="PSUM"))
                    ps_av = st3.enter_context(
                        tc.tile_pool(name="ps_av3", bufs=1, space="PSUM"))
                    scr3 = st3.enter_context(
                        tc.tile_pool(name="scr3", bufs=2, space="PSUM"))
                    ex = st3.enter_context(tc.tile_pool(name="ex3", bufs=3))
                    on = st3.enter_context(tc.tile_pool(name="on3", bufs=4))
                    ozs = st3.enter_context(tc.tile_pool(name="ozs3", bufs=4))
                    osb = st3.enter_context(tc.tile_pool(name="osb3", bufs=2))

                    def attn_u3(j, u, ss):
                        p_s2 = ps_s.tile([128, 1024], f32, tag="p_s2")
                        us = slice(u * 128, (u + 1) * 128)
                        nc.tensor.matmul(p_s2[:, 0:512],
                                         kfT[0:64, j, us],
                                         qfT[0:64, j, ss],
                                         start=True, stop=True)
                        nc.tensor.matmul(p_s2[:, 512:1024],
                                         kfT[64:128, j, us],
                                         qfT[64:128, j, ss],
                                         start=True, stop=True)
                        e2 = ex.tile([128, 1024], bf16, tag="e2")
                        nc.scalar.activation(e2[:], p_s2[:], AF.Exp,
                                             scale=0.125)
                        return e2

                    def norms3(onj, p_avA, p_avB):
                        for half, p_av in ((0, p_avA), (64, p_avB)):
                            hs = slice(half, half + 64)
                            avr = ozs.tile([65, 512], f32, tag="avr",
                                           bufs=2)
                            nc.vector.tensor_copy(avr[:], p_av[0:65, :])
                            rz = ozs.tile([1, 512], f32, tag="rz")
                            with nc.allow_low_precision(reason="bf16 Z"):
                                nc.vector.reciprocal(rz[:], avr[64:65, :])
                            zb = ozs.tile([64, 512], f32, tag="zb")
                            nc.gpsimd.partition_broadcast(zb[:], rz[:],
                                                          channels=64)
                            nc.vector.tensor_mul(onj[hs, :],
                                                 avr[0:64, :], zb[:])

                    def outproj_chunk3(sb_i, onorm, tc_i, ei):
                        es = slice(ei * 512, (ei + 1) * 512)
                        tcs = slice(tc_i * 128, (tc_i + 1) * 128)
                        p_o = scr3.tile([128, 512], f32, tag="scr",
                                        name=f"p_o_{sb_i}_{tc_i}_{ei}")
                        for kk in range(2):
                            nc.tensor.matmul(
                                p_o[:], onorm[kk][:, tcs],
                                wout_b[:, kk, es],
                                start=(kk == 0), stop=(kk == 1))
                        o_t = osb.tile([128, 512], bf16, tag="o_t",
                                       name=f"o_t_{sb_i}_{tc_i}_{ei}")
                        nc.vector.tensor_copy(o_t[:], p_o[:])
                        nc.sync.dma_start(
                            out_d[sb_i * 512 + tc_i * 128:
                                  sb_i * 512 + tc_i * 128 + 128, es],
                            o_t[:])

                    # remaining (sb, j) pairs: (0,1), (1,0), (1,1), ...,
                    # with fills: outproj of the last finished sb + q pairs
                    # of sb_i+1 interleaved into the u-loops.
                    prev = [None]
                    for sb_i in range(SB):
                        ss = slice(sb_i * 512, (sb_i + 1) * 512)
                        if sb_i == 0:
                            onorm = onorm0
                        else:
                            onorm = [on.tile([128, 512], bf16, tag=f"on{j}",
                                             name=f"on{j}_{sb_i}")
                                     for j in range(2)]
                        fl = []
                        if prev[0] is not None:
                            p_sb, p_on = prev[0]
                            for tc_i in range(4):
                                for ei in range(2):
                                    fl.append(
                                        lambda t=tc_i, e=ei, s=p_sb, o=p_on:
                                        outproj_chunk3(s, o, t, e))
                        if sb_i + 1 < SB:
                            for tp in (2 * (sb_i + 1), 2 * (sb_i + 1) + 1):
                                st_ = {}

                                def mk_mm(tp=tp, st_=st_):
                                    st_["qf"] = q_pair_mm(tp, scr3,
                                                          tag="scr")

                                def mk_tp(tp=tp, st_=st_):
                                    q_pair_tp(tp, scr3, st_["qf"],
                                              tag="scr")
                                fl.append(mk_mm)
                                fl.append(mk_tp)
                        fi = 0
                        js_here = [1] if sb_i == 0 else [0, 1]
                        nstep = len(js_here) * UT
                        step = 0
                        for j in js_here:
                            hA, hB = 2 * j, 2 * j + 1
                            p_avA = ps_av.tile([128, 512], f32, tag="p_avA",
                                               name=f"p_avA3_{sb_i}_{j}")
                            p_avB = ps_av.tile([128, 512], f32, tag="p_avB",
                                               name=f"p_avB3_{sb_i}_{j}")
                            for u in range(UT):
                                e2 = attn_u3(j, u, ss)
                                step += 1
                                want = (step * len(fl)) // nstep
                                while fi < want:
                                    fl[fi]()
                                    fi += 1
                                nc.tensor.matmul(
                                    p_avA[0:65, :],
                                    kvd[u][:, hA * 97 + 32:hA * 97 + 97],
                                    e2[:, 0:512],
                                    start=(u == 0), stop=(u == UT - 1))
                                nc.tensor.matmul(
                                    p_avB[0:65, :],
                                    kvd[u][:, hB * 97 + 32:hB * 97 + 97],
                                    e2[:, 512:1024],
                                    start=(u == 0), stop=(u == UT - 1))
                            norms3(onorm[j], p_avA, p_avB)
                        while fi < len(fl):
                            fl[fi]()
                            fi += 1
                        prev[0] = (sb_i, onorm)

                    # final outproj for last sb
                    p_sb, p_on = prev[0]
                    for tc_i in range(4):
                        for ei in range(2):
                            outproj_chunk3(p_sb, p_on, tc_i, ei)

            if reps == 1:
                body()
            else:
                with tc.For_i(0, reps, 1):
                    body()

    nc.compile()
    return nc


def _host_prep(x, Wqa, g_qa, b_qa, Wqb, Wkva, g_kva, b_kva, Wkvb, Wout):
    import ml_dtypes
    f32 = np.float32
    bf16 = ml_dtypes.bfloat16
    x = np.asarray(x, f32)
    Wqa = np.asarray(Wqa, f32); Wqb = np.asarray(Wqb, f32)
    Wkva = np.asarray(Wkva, f32); Wkvb = np.asarray(Wkvb, f32)
    Wout = np.asarray(Wout, f32)
    g_qa = np.asarray(g_qa, f32); b_qa = np.asarray(b_qa, f32)
    g_kva = np.asarray(g_kva, f32); b_kva = np.asarray(b_kva, f32)

    inv = 1.0 / (10000.0 ** (np.arange(0, ROPE, 2, dtype=f32) / ROPE))
    fr = np.arange(S, dtype=f32)[:, None] * inv[None, :]
    cos, sin = np.cos(fr).astype(f32), np.sin(fr).astype(f32)
    c2 = np.repeat(cos, 2, axis=1)
    s2 = np.empty((S, ROPE), f32)
    s2[:, 0::2] = -sin
    s2[:, 1::2] = sin
    Aq = np.ones((S, DPC), f32)
    Bq = np.zeros((S, DPC), f32)
    for h in range(HPC):
        Aq[:, h * 64 + 32:h * 64 + 64] = c2
        Bq[:, h * 64 + 32:h * 64 + 64] = s2

    shared = {
        "WqaT": np.ascontiguousarray(Wqa.T).astype(bf16),
        "WkvaT": np.ascontiguousarray(Wkva.T).astype(bf16),
        "Aq": Aq.astype(bf16), "Bq": Bq.astype(bf16),
        "c2k": c2.astype(bf16), "s2k": s2.astype(bf16),
        "ident": np.eye(128, dtype=f32).astype(bf16),
        "ones1": np.ones((1, 128), f32).astype(bf16),
        "epst": np.full((128, 1), EPS, f32),
    }
    in_maps = []
    for core in range(NCORES):
        b, hg = core // HPC, core % HPC
        Wqb_sl = Wqb[hg * DPC:(hg + 1) * DPC, :]
        WkvbT_eff = np.zeros((KVL, KVW), f32)
        kconst = np.zeros((1, KVW), f32)
        for h in range(HPC):
            blk = Wkvb[(hg * HPC + h) * 96:(hg * HPC + h + 1) * 96, :] \
                * g_kva[None, :]
            WkvbT_eff[:, h * 97:h * 97 + 96] = blk.T
            kconst[0, h * 97:h * 97 + 96] = b_kva @ blk.T
            kconst[0, h * 97 + 96] = 1.0
        m = dict(shared)
        m["xT"] = np.ascontiguousarray(x[b].T).astype(bf16)
        m["WqbT"] = np.ascontiguousarray(
            (Wqb_sl * g_qa[None, :]).T).astype(bf16)
        m["qconst"] = (b_qa @ Wqb_sl.T)[None, :].astype(bf16)
        m["WkvbT"] = WkvbT_eff.astype(bf16)
        m["kconst"] = kconst.astype(bf16)
        m["WoutT"] = np.ascontiguousarray(
            Wout[:, hg * DPC:(hg + 1) * DPC].T).astype(bf16)
        in_maps.append(m)
    return in_maps


def kernel(**inputs):
    from concourse.bass_utils import run_bass_kernel_spmd
    if "nc" not in _CACHE:
        _CACHE["nc"] = _build(reps=1)
    nc = _CACHE["nc"]
    in_maps = _host_prep(**inputs)
    res = run_bass_kernel_spmd(nc, in_maps, core_ids=list(range(NCORES)))
    out = np.zeros((B, S, E), np.float32)
    for core in range(NCORES):
        out[core // HPC] += res.results[core]["out"].astype(np.float32)
    return out


# revision 15
# speedup vs baseline: 1.0479x; 1.0479x over previous
"""MLA encoder self-attention on 8 TRN2 NeuronCores.

Sharding: data-parallel over batch (B=2) x tensor-parallel over head groups
(16 heads -> 4 groups of 4). Core c handles batch c//4, heads 4*(c%4)..+4.
Each core computes its heads' attention and a partial output projection;
the host sums the 4 head-group partials per batch.

v3 (vs v1 baseline):
- All DRAM traffic and matmul operands are bf16 (halves DMA, enables PE
  fast-weight-load); accumulation stays fp32 in PSUM.
- Activation engine runs only LN/Sqrt/Exp (Exp is its serial floor in
  the attention phase); PSUM evacuations split between ACT (stage 1/2)
  and DVE (attention phase).
- DMA queues: x on sync, stage-1 weights on scalar/gpsimd FIRST, then
  stage-2 weights + Aq/Bq rope tables prefetched behind them.
- The q decompress+rope path runs in token-tile pairs; its matmuls and
  transposes are separate fill units interleaved (with out-projection
  chunks) into the attention u-loops, so PE fills the exp-wait gaps.
  All fills share one [128,512]-f32 PSUM scratch tag (2 banks), fitting
  the 8-bank budget with double-buffered scores (4) + AV accumulators
  (2).
- Output is written bf16; host accumulates partials in fp32.

Key performance structure (unchanged from v1):
- Attention scores for the two heads of a pair use K=64 contractions at
  PE row-groups (0,*) and (64,*) issued back-to-back.
- The two heads' score tiles land in one [128,1024] PSUM tile, so one
  Exp activation covers both; softmax denominator comes from an
  appended ones-column in kvd (row 64 of the AV accumulation),
  normalization deferred to right before the output projection.
"""
import numpy as np

B, S, E = 2, 2048, 1024
H, HD = 16, 64
ROPE, NOPE = 32, 32
QL, KVL = 512, 512
EPS = 1e-5
HPC = 4                # heads per core
DPC = HPC * HD         # 256
NCORES = 8
TT = S // 128          # 16 token tiles
ET = E // 128          # 8
RT = QL // 128         # 4
SB = S // 512          # 4 s-blocks
UT = S // 128          # 16 key tiles
KVW = 4 * 97           # 388: kvd width

_CACHE = {}


def _build(reps=1):
    import concourse.tile as tile
    import concourse.mybir as mybir
    from concourse import bacc

    f32 = mybir.dt.float32
    bf16 = mybir.dt.bfloat16
    AF = mybir.ActivationFunctionType
    ALU = mybir.AluOpType

    nc = bacc.Bacc("TRN2", target_bir_lowering=False, debug=False,
                   num_devices=NCORES)

    def din(name, shape, dt=bf16):
        return nc.dram_tensor(name, shape, dt, kind="ExternalInput").ap()

    xT_d = din("xT", (E, S))
    wqa_d = din("WqaT", (E, QL))
    wkva_d = din("WkvaT", (E, KVL + ROPE))
    wqb_d = din("WqbT", (QL, DPC))
    qconst_d = din("qconst", (1, DPC))
    wkvb_d = din("WkvbT", (KVL, KVW))
    kconst_d = din("kconst", (1, KVW))
    wout_d = din("WoutT", (DPC, E))
    aq_d = din("Aq", (S, DPC))
    bq_d = din("Bq", (S, DPC))
    c2k_d = din("c2k", (S, ROPE))
    s2k_d = din("s2k", (S, ROPE))
    ident_d = din("ident", (128, 128))
    ones1_d = din("ones1", (1, 128))
    eps_d = din("epst", (128, 1), mybir.dt.float32)
    out_d = nc.dram_tensor("out", (S, E), bf16, kind="ExternalOutput").ap()

    with tile.TileContext(nc) as tc:
        import contextlib
        with contextlib.ExitStack() as top:
            consts = top.enter_context(tc.tile_pool(name="consts", bufs=1))
            ident_b = consts.tile([128, 128], bf16, tag="ident_b")
            nc.sync.dma_start(ident_b[:], ident_d[:])
            ones1_b = consts.tile([1, 128], bf16, tag="ones1")
            nc.sync.dma_start(ones1_b[:], ones1_d[:])
            qconst_b = consts.tile([1, DPC], bf16, tag="qconst")
            nc.sync.dma_start(qconst_b[:], qconst_d[:])
            kconst_b = consts.tile([1, KVW], bf16, tag="kconst")
            nc.sync.dma_start(kconst_b[:], kconst_d[:])
            eps_t = consts.tile([128, 1], f32, tag="epst")
            nc.sync.dma_start(eps_t[:], eps_d[:])

            acts = top.enter_context(tc.tile_pool(name="acts", bufs=1))
            wq2 = top.enter_context(tc.tile_pool(name="wq2", bufs=1))
            wqa_p = top.enter_context(tc.tile_pool(name="wqa", bufs=1))
            ck = top.enter_context(tc.tile_pool(name="ck", bufs=1))
            qp = top.enter_context(tc.tile_pool(name="qp", bufs=2))

            def body():
                # feature-major stage-1 outputs: [128, RT, S] (r on mid dim)
                qcnT = acts.tile([128, RT, S], bf16, tag="qcnT", name="qcnT")
                ckvnT = acts.tile([128, RT, S], bf16, tag="ckvnT",
                                  name="ckvnT")
                kpe = [acts.tile([128, ROPE], bf16, tag=f"kpe{t}",
                                 name=f"kpe{t}")
                       for t in range(TT)]

                # stage-1 weights first on the scalar/gpsimd queues (the
                # first matmuls wait on e-slice 0)
                wqa_t = wqa_p.tile([128, ET, QL], bf16, tag="wqa")
                wkva_t = wqa_p.tile([128, ET, KVL + ROPE], bf16, tag="wkva")
                for e in range(ET):
                    es_ = slice(e * 128, (e + 1) * 128)
                    nc.scalar.dma_start(wqa_t[:, e, :], wqa_d[es_, :])
                    nc.gpsimd.dma_start(wkva_t[:, e, :], wkva_d[es_, :])
                c2k_t = ck.tile([128, TT, ROPE], bf16, tag="c2k")
                nc.scalar.dma_start(
                    c2k_t[:], c2k_d.rearrange("(t p) r -> p t r", p=128))
                s2k_t = ck.tile([128, TT, ROPE], bf16, tag="s2k")
                nc.scalar.dma_start(
                    s2k_t[:], s2k_d.rearrange("(t p) r -> p t r", p=128))

                # stage-2 weights + rope tables prefetch behind them; they
                # land while stage 1 computes.
                wqb_b = wq2.tile([128, RT, DPC], bf16, tag="wqb")
                nc.scalar.dma_start(
                    wqb_b[:], wqb_d.rearrange("(k p) n -> p k n", p=128))
                wkvb_b = wq2.tile([128, RT, KVW], bf16, tag="wkvb")
                nc.scalar.dma_start(
                    wkvb_b[:], wkvb_d.rearrange("(k p) n -> p k n", p=128))
                wout_b = wq2.tile([128, 2, E], bf16, tag="wout")
                nc.scalar.dma_start(
                    wout_b[:], wout_d.rearrange("(k p) n -> p k n", p=128))
                aq_t = wq2.tile([128, TT, DPC], bf16, tag="aq")
                nc.gpsimd.dma_start(
                    aq_t[:], aq_d.rearrange("(t p) d -> p t d", p=128))
                bq_t = wq2.tile([128, TT, DPC], bf16, tag="bq")
                nc.gpsimd.dma_start(
                    bq_t[:], bq_d.rearrange("(t p) d -> p t d", p=128))

                # ---------------- stage 1: qa + kva + LN + kpe rope -------
                with contextlib.ExitStack() as st1:
                    pmm = st1.enter_context(
                        tc.tile_pool(name="pmm", bufs=2, space="PSUM"))
                    ptp = st1.enter_context(
                        tc.tile_pool(name="ptp", bufs=1, space="PSUM"))
                    xp = st1.enter_context(tc.tile_pool(name="xp", bufs=3))
                    sp = st1.enter_context(tc.tile_pool(name="sp", bufs=3))

                    pend = []

                    def do_tp1(t, qcn_t, ckvn_t):
                        ts_ = slice(t * 128, (t + 1) * 128)
                        tp = ptp.tile([128, 2 * QL], bf16, tag="tp1",
                                      bufs=2, name=f"tp1_{t}")
                        tpq, tpk = tp[:, 0:QL], tp[:, QL:2 * QL]
                        for r in range(RT):
                            rs = slice(r * 128, (r + 1) * 128)
                            nc.tensor.transpose(tpq[:, rs], qcn_t[:, rs],
                                                ident_b[:])
                            nc.tensor.transpose(tpk[:, rs], ckvn_t[:, rs],
                                                ident_b[:])
                        nc.scalar.activation(
                            qcnT[:, :, ts_],
                            tpq.rearrange("p (r c) -> p r c", r=RT)[:],
                            AF.Copy)
                        nc.vector.tensor_copy(
                            ckvnT[:, :, ts_],
                            tpk.rearrange("p (r c) -> p r c", r=RT)[:])

                    for t in range(TT):
                        ts_ = slice(t * 128, (t + 1) * 128)
                        p_qc = pmm.tile([128, QL], f32, tag="p_qc")
                        p_ka = pmm.tile([128, 272], f32, tag="p_ka")
                        p_kb = pmm.tile([128, 272], f32, tag="p_kb")
                        xt = xp.tile([128, ET, 128], bf16, tag="xt")
                        if t == 0:
                            for e in range(ET):
                                es_ = slice(e * 128, (e + 1) * 128)
                                nc.sync.dma_start(
                                    xt[:, e, :], xT_d[es_, ts_])
                        else:
                            nc.sync.dma_start(
                                xt[:],
                                xT_d.rearrange("(k p) s -> p k s",
                                               p=128)[:, :, ts_])
                        for e in range(ET):
                            st, sp_ = (e == 0), (e == ET - 1)
                            nc.tensor.matmul(p_qc[:], xt[:, e, :],
                                             wqa_t[:, e, :],
                                             start=st, stop=sp_)
                            nc.tensor.matmul(p_ka[:], xt[:, e, :],
                                             wkva_t[:, e, 0:272],
                                             start=st, stop=sp_)
                            nc.tensor.matmul(p_kb[:], xt[:, e, :],
                                             wkva_t[:, e, 272:544],
                                             start=st, stop=sp_)
                        # LN on qc
                        stq = sp.tile([128, 6], f32, tag="stq")
                        nc.vector.bn_stats(stq[:], p_qc[:])
                        mvq = sp.tile([128, 2], f32, tag="mvq")
                        nc.vector.bn_aggr(mvq[:], stq[:])
                        rsq = sp.tile([128, 1], f32, tag="rsq")
                        nc.scalar.activation(rsq[:], mvq[:, 1:2], AF.Sqrt,
                                             bias=eps_t[:])
                        rsq2 = sp.tile([128, 1], f32, tag="rsq2")
                        nc.vector.reciprocal(rsq2[:], rsq[:])
                        nmq = sp.tile([128, 1], f32, tag="nmq")
                        nc.vector.tensor_scalar(
                            out=nmq[:], in0=mvq[:, 0:1], scalar1=rsq2[:],
                            scalar2=-1.0, op0=ALU.mult, op1=ALU.mult)
                        qcn_t = sp.tile([128, QL], bf16, tag="qcn", bufs=3)
                        nc.scalar.activation(qcn_t[:], p_qc[:], AF.Identity,
                                             bias=nmq[:], scale=rsq2[:])
                        # LN on ckv (272 + 240 chunks)
                        stk = sp.tile([128, 2, 6], f32, tag="stk")
                        nc.vector.bn_stats(stk[:, 0, :], p_ka[:])
                        nc.vector.bn_stats(stk[:, 1, :], p_kb[:, 0:240])
                        mvk = sp.tile([128, 2], f32, tag="mvk")
                        nc.vector.bn_aggr(mvk[:], stk[:])
                        rsk = sp.tile([128, 1], f32, tag="rsk")
                        nc.scalar.activation(rsk[:], mvk[:, 1:2], AF.Sqrt,
                                             bias=eps_t[:])
                        rsk2 = sp.tile([128, 1], f32, tag="rsk2")
                        nc.vector.reciprocal(rsk2[:], rsk[:])
                        nmk = sp.tile([128, 1], f32, tag="nmk")
                        nc.vector.tensor_scalar(
                            out=nmk[:], in0=mvk[:, 0:1], scalar1=rsk2[:],
                            scalar2=-1.0, op0=ALU.mult, op1=ALU.mult)
                        ckvn_t = sp.tile([128, KVL], bf16, tag="ckvn", bufs=3)
                        nc.scalar.activation(ckvn_t[:, 0:272], p_ka[:],
                                             AF.Identity, bias=nmk[:],
                                             scale=rsk2[:])
                        nc.scalar.activation(ckvn_t[:, 272:512], p_kb[:, 0:240],
                                             AF.Identity, bias=nmk[:],
                                             scale=rsk2[:])
                        # kpe rope (raw cols 240:272 of p_kb)
                        kraw = sp.tile([128, ROPE], bf16, tag="kraw")
                        nc.vector.tensor_copy(kraw[:], p_kb[:, 240:272])
                        ksw = sp.tile([128, ROPE], bf16, tag="ksw")
                        kraw3 = kraw.rearrange("p (i two) -> p i two", two=2)
                        ksw3 = ksw.rearrange("p (i two) -> p i two", two=2)
                        nc.gpsimd.tensor_copy(ksw3[:, :, 0:1], kraw3[:, :, 1:2])
                        nc.gpsimd.tensor_copy(ksw3[:, :, 1:2], kraw3[:, :, 0:1])
                        c2t = c2k_t[:, t, :]
                        s2t = s2k_t[:, t, :]
                        kp1 = sp.tile([128, ROPE], bf16, tag="kp1")
                        nc.gpsimd.tensor_mul(kp1[:], kraw[:], c2t[:])
                        kp2 = sp.tile([128, ROPE], bf16, tag="kp2")
                        nc.gpsimd.tensor_mul(kp2[:], ksw[:], s2t[:])
                        nc.gpsimd.tensor_add(kpe[t][:], kp1[:], kp2[:])
                        pend.append((t, qcn_t, ckvn_t))
                        if len(pend) > 2:
                            do_tp1(*pend.pop(0))
                    for args in pend:
                        do_tp1(*args)

                # ---------------- q path (pairs of token tiles) -----------
                acts2 = top.enter_context(tc.tile_pool(name="acts2", bufs=1))
                qfT = acts2.tile([128, 2, S], bf16, tag="qfT", name="qfT")
                kfT = acts2.tile([128, 2, S], bf16, tag="kfT", name="kfT")
                kvd = [acts2.tile([128, KVW], bf16, tag=f"kvd{t}",
                                  name=f"kvd{t}")
                       for t in range(TT)]

                def q_pair_mm(tp, pool):
                    """q decompress + rope for token tiles 2tp, 2tp+1.
                    Returns qf_t for the later transpose fill unit."""
                    t0 = 2 * tp
                    p_q = pool.tile([128, 512], f32, tag="scr",
                                    name=f"p_q_{tp}")
                    for half in range(2):
                        hs = slice(half * DPC, (half + 1) * DPC)
                        hts = slice((t0 + half) * 128, (t0 + half + 1) * 128)
                        for r in range(RT):
                            nc.tensor.matmul(p_q[:, hs], qcnT[:, r, hts],
                                             wqb_b[:, r, :],
                                             start=(r == 0), stop=False)
                        nc.tensor.matmul(p_q[:, hs], ones1_b[:], qconst_b[:],
                                         start=False, stop=True)
                    q_t = qp.tile([128, 512], bf16, tag="q_t")
                    nc.vector.tensor_copy(q_t[:], p_q[:])
                    # rope (Aq/Bq carry cos/+-sin on rope cols, 1/0 on nope)
                    a_t = aq_t[:, t0:t0 + 2, :].rearrange("p t d -> p (t d)")
                    b_t = bq_t[:, t0:t0 + 2, :].rearrange("p t d -> p (t d)")
                    q_sw = qp.tile([128, 512], bf16, tag="q_sw")
                    q3 = q_t.rearrange("p (i two) -> p i two", two=2)
                    qs3 = q_sw.rearrange("p (i two) -> p i two", two=2)
                    nc.gpsimd.tensor_copy(qs3[:, :, 0:1], q3[:, :, 1:2])
                    nc.gpsimd.tensor_copy(qs3[:, :, 1:2], q3[:, :, 0:1])
                    qt1 = qp.tile([128, 512], bf16, tag="qt1")
                    nc.vector.tensor_mul(qt1[:], q_t[:], a_t)
                    qt2 = qp.tile([128, 512], bf16, tag="qt2")
                    nc.vector.tensor_mul(qt2[:], q_sw[:], b_t)
                    qf_t = qp.tile([128, 512], bf16, tag="qf_t", bufs=3)
                    nc.vector.tensor_add(qf_t[:], qt1[:], qt2[:])
                    return qf_t

                def q_pair_tp(tp, pool, qf_t):
                    """Transpose fill unit: qf pair -> qfT (emitted a few
                    u-steps after q_pair_mm so rope has completed)."""
                    t0 = 2 * tp
                    ss = slice(t0 * 128, (t0 + 2) * 128)
                    tpq = pool.tile([128, 512], bf16, tag="scr",
                                    name=f"tpq2_{tp}")
                    for c in range(4):
                        cs = slice(c * 128, (c + 1) * 128)
                        nc.tensor.transpose(tpq[:, cs], qf_t[:, cs],
                                            ident_b[:])
                    # tpq cols: (t0 j0, t0 j1, t1 j0, t1 j1)
                    nc.vector.tensor_copy(
                        qfT[:, :, ss].rearrange("p j (t c) -> p j t c", t=2),
                        tpq.rearrange("p (t j c) -> p j t c", t=2, j=2)[:])

                # ---------------- stage 2: kvb + k assembly ---------------
                with contextlib.ExitStack() as st2:
                    pkv = st2.enter_context(
                        tc.tile_pool(name="pkv", bufs=2, space="PSUM"))
                    ptp2 = st2.enter_context(
                        tc.tile_pool(name="ptp2", bufs=2, space="PSUM"))
                    ptpq0 = st2.enter_context(
                        tc.tile_pool(name="ptpq0", bufs=2, space="PSUM"))
                    qk2 = st2.enter_context(tc.tile_pool(name="qk2", bufs=2))

                    pend2 = []

                    def do_tpk(t, kf_t):
                        ts_ = slice(t * 128, (t + 1) * 128)
                        tpk = ptp2.tile([128, DPC], bf16, tag="tp2k",
                                        name=f"tpk2_{t}")
                        for j in range(2):
                            js = slice(j * 128, (j + 1) * 128)
                            nc.tensor.transpose(tpk[:, js], kf_t[:, js],
                                                ident_b[:])
                        nc.vector.tensor_copy(
                            kfT[:, :, ts_],
                            tpk.rearrange("p (j c) -> p j c", j=2)[:])

                    qf01 = {}
                    for t in range(TT):
                        ts_ = slice(t * 128, (t + 1) * 128)
                        p_kv = pkv.tile([128, KVW], f32, tag="p_kv")
                        for r in range(RT):
                            nc.tensor.matmul(p_kv[:], ckvnT[:, r, ts_],
                                             wkvb_b[:, r, :],
                                             start=(r == 0), stop=False)
                        # kconst has 1.0 at the ones positions (col 96 of
                        # each head) so kvd's softmax-denominator column
                        # needs no separate write.
                        nc.tensor.matmul(p_kv[:], ones1_b[:], kconst_b[:],
                                         start=False, stop=True)
                        nc.scalar.activation(kvd[t][:], p_kv[:], AF.Copy)
                        # k_full assembly: nope part from kvd (SBUF, Pool),
                        # rope part broadcast from kpe.
                        kf_t = qk2.tile([128, DPC], bf16, tag="kf_t", bufs=3)
                        kf3 = kf_t.rearrange("p (h c) -> p h c", h=HPC)
                        kvd3 = kvd[t].rearrange("p (h c) -> p h c", h=HPC)
                        nc.gpsimd.tensor_copy(kf3[:, :, 0:32],
                                              kvd3[:, :, 0:32])
                        for h in range(HPC):
                            nc.gpsimd.tensor_copy(
                                kf_t[:, h * 64 + 32:h * 64 + 64], kpe[t][:])
                        pend2.append((t, kf_t))
                        if len(pend2) > 2:
                            do_tpk(*pend2.pop(0))
                        # q path for sb 0 rides along in spare PE slots
                        if t == 4:
                            qf01[0] = q_pair_mm(0, ptpq0)
                        elif t == 6:
                            q_pair_tp(0, ptpq0, qf01[0])
                        elif t == 8:
                            qf01[1] = q_pair_mm(1, ptpq0)
                        elif t == 10:
                            q_pair_tp(1, ptpq0, qf01[1])
                    for args in pend2:
                        do_tpk(*args)

                # ---------------- stage 3: attention + fills --------------
                with contextlib.ExitStack() as st3:
                    ps_s = st3.enter_context(
                        tc.tile_pool(name="ps_s", bufs=2, space="PSUM"))
                    ps_av = st3.enter_context(
                        tc.tile_pool(name="ps_av", bufs=1, space="PSUM"))
                    ex = st3.enter_context(tc.tile_pool(name="ex", bufs=3))
                    on = st3.enter_context(tc.tile_pool(name="on", bufs=4))
                    ozs = st3.enter_context(tc.tile_pool(name="ozs", bufs=4))
                    osb = st3.enter_context(tc.tile_pool(name="osb", bufs=2))

                    def outproj_chunk(pool, sb_i, onorm, tc_i, ei):
                        es = slice(ei * 512, (ei + 1) * 512)
                        tcs = slice(tc_i * 128, (tc_i + 1) * 128)
                        p_o = pool.tile([128, 512], f32, tag="scr",
                                        name=f"p_o_{sb_i}_{tc_i}_{ei}")
                        for kk in range(2):
                            nc.tensor.matmul(
                                p_o[:], onorm[kk][:, tcs],
                                wout_b[:, kk, es],
                                start=(kk == 0), stop=(kk == 1))
                        o_t = osb.tile([128, 512], bf16, tag="o_t",
                                       name=f"o_t_{sb_i}_{tc_i}_{ei}")
                        nc.vector.tensor_copy(o_t[:], p_o[:])
                        nc.sync.dma_start(
                            out_d[sb_i * 512 + tc_i * 128:
                                  sb_i * 512 + tc_i * 128 + 128, es],
                            o_t[:])

                    prev = [None]
                    for sb_i in range(SB):
                        ss = slice(sb_i * 512, (sb_i + 1) * 512)
                        onorm = [on.tile([128, 512], bf16, tag=f"on{j}",
                                         name=f"on{j}_{sb_i}")
                                 for j in range(2)]
                        # fill units for this sb's u-loops: outproj of
                        # sb_i-1 and q path of sb_i+1, all sharing one
                        # [128,512]-f32 PSUM scratch tag (2 banks).
                        win = contextlib.ExitStack()
                        scr = win.enter_context(tc.tile_pool(
                            name=f"scr_{sb_i}", bufs=2, space="PSUM"))
                        fl = []
                        if prev[0] is not None:
                            p_sb, p_on = prev[0]
                            for tc_i in range(4):
                                for ei in range(2):
                                    fl.append(
                                        lambda t=tc_i, e=ei, s=p_sb, o=p_on:
                                        outproj_chunk(scr, s, o, t, e))
                        if sb_i + 1 < SB:
                            for tp in (2 * (sb_i + 1), 2 * (sb_i + 1) + 1):
                                st_ = {}

                                def mk_mm(tp=tp, st_=st_):
                                    st_["qf"] = q_pair_mm(tp, scr)

                                def mk_tp(tp=tp, st_=st_):
                                    q_pair_tp(tp, scr, st_["qf"])
                                fl.append(mk_mm)
                                fl.append(mk_tp)
                        fi = 0
                        nstep = 2 * UT
                        step = 0
                        for j in range(2):
                            hA, hB = 2 * j, 2 * j + 1
                            p_avA = ps_av.tile([128, 512], f32, tag="p_avA",
                                               name=f"p_avA_{sb_i}_{j}")
                            p_avB = ps_av.tile([128, 512], f32, tag="p_avB",
                                               name=f"p_avB_{sb_i}_{j}")
                            for u in range(UT):
                                us = slice(u * 128, (u + 1) * 128)
                                p_s2 = ps_s.tile([128, 1024], f32, tag="p_s2")
                                # two heads' scores: K=64 row-groups (0,*)
                                # and (64,*) run concurrently on the PE
                                nc.tensor.matmul(p_s2[:, 0:512],
                                                 kfT[0:64, j, us],
                                                 qfT[0:64, j, ss],
                                                 start=True, stop=True)
                                nc.tensor.matmul(p_s2[:, 512:1024],
                                                 kfT[64:128, j, us],
                                                 qfT[64:128, j, ss],
                                                 start=True, stop=True)
                                e2 = ex.tile([128, 1024], bf16, tag="e2")
                                nc.scalar.activation(e2[:], p_s2[:], AF.Exp,
                                                     scale=0.125)
                                # fills run during the exp window
                                step += 1
                                want = (step * len(fl)) // nstep
                                while fi < want:
                                    fl[fi]()
                                    fi += 1
                                nc.tensor.matmul(
                                    p_avA[0:65, :],
                                    kvd[u][:, hA * 97 + 32:hA * 97 + 97],
                                    e2[:, 0:512],
                                    start=(u == 0), stop=(u == UT - 1))
                                nc.tensor.matmul(
                                    p_avB[0:65, :],
                                    kvd[u][:, hB * 97 + 32:hB * 97 + 97],
                                    e2[:, 512:1024],
                                    start=(u == 0), stop=(u == UT - 1))
                            for half, p_av in ((0, p_avA), (64, p_avB)):
                                hs = slice(half, half + 64)
                                rz = ozs.tile([1, 512], f32, tag="rz")
                                with nc.allow_low_precision(reason="bf16 Z"):
                                    nc.vector.reciprocal(rz[:],
                                                         p_av[64:65, :])
                                zb = ozs.tile([64, 512], f32, tag="zb")
                                nc.gpsimd.partition_broadcast(zb[:], rz[:],
                                                              channels=64)
                                nc.vector.tensor_mul(onorm[j][hs, :],
                                                     p_av[0:64, :], zb[:])
                        while fi < len(fl):
                            fl[fi]()
                            fi += 1
                        win.close()
                        prev[0] = (sb_i, onorm)

                    # final outproj for last sb
                    with tc.tile_pool(name="ps_of", bufs=2,
                                      space="PSUM") as ps_of:
                        p_sb, p_on = prev[0]
                        for tc_i in range(4):
                            for ei in range(2):
                                outproj_chunk(ps_of, p_sb, p_on, tc_i, ei)

            if reps == 1:
                body()
            else:
                with tc.For_i(0, reps, 1):
                    body()

    nc.compile()
    return nc


def _host_prep(x, Wqa, g_qa, b_qa, Wqb, Wkva, g_kva, b_kva, Wkvb, Wout):
    import ml_dtypes
    f32 = np.float32
    bf16 = ml_dtypes.bfloat16
    x = np.asarray(x, f32)
    Wqa = np.asarray(Wqa, f32); Wqb = np.asarray(Wqb, f32)
    Wkva = np.asarray(Wkva, f32); Wkvb = np.asarray(Wkvb, f32)
    Wout = np.asarray(Wout, f32)
    g_qa = np.asarray(g_qa, f32); b_qa = np.asarray(b_qa, f32)
    g_kva = np.asarray(g_kva, f32); b_kva = np.asarray(b_kva, f32)

    inv = 1.0 / (10000.0 ** (np.arange(0, ROPE, 2, dtype=f32) / ROPE))
    fr = np.arange(S, dtype=f32)[:, None] * inv[None, :]
    cos, sin = np.cos(fr).astype(f32), np.sin(fr).astype(f32)
    c2 = np.repeat(cos, 2, axis=1)
    s2 = np.empty((S, ROPE), f32)
    s2[:, 0::2] = -sin
    s2[:, 1::2] = sin
    Aq = np.ones((S, DPC), f32)
    Bq = np.zeros((S, DPC), f32)
    for h in range(HPC):
        Aq[:, h * 64 + 32:h * 64 + 64] = c2
        Bq[:, h * 64 + 32:h * 64 + 64] = s2

    shared = {
        "WqaT": np.ascontiguousarray(Wqa.T).astype(bf16),
        "WkvaT": np.ascontiguousarray(Wkva.T).astype(bf16),
        "Aq": Aq.astype(bf16), "Bq": Bq.astype(bf16),
        "c2k": c2.astype(bf16), "s2k": s2.astype(bf16),
        "ident": np.eye(128, dtype=f32).astype(bf16),
        "ones1": np.ones((1, 128), f32).astype(bf16),
        "epst": np.full((128, 1), EPS, f32),
    }
    in_maps = []
    for core in range(NCORES):
        b, hg = core // HPC, core % HPC
        Wqb_sl = Wqb[hg * DPC:(hg + 1) * DPC, :]
        WkvbT_eff = np.zeros((KVL, KVW), f32)
        kconst = np.zeros((1, KVW), f32)
        for h in range(HPC):
            blk = Wkvb[(hg * HPC + h) * 96:(hg * HPC + h + 1) * 96, :] \
                * g_kva[None, :]
            WkvbT_eff[:, h * 97:h * 97 + 96] = blk.T
            kconst[0, h * 97:h * 97 + 96] = b_kva @ blk.T
            kconst[0, h * 97 + 96] = 1.0
        m = dict(shared)
        m["xT"] = np.ascontiguousarray(x[b].T).astype(bf16)
        m["WqbT"] = np.ascontiguousarray(
            (Wqb_sl * g_qa[None, :]).T).astype(bf16)
        m["qconst"] = (b_qa @ Wqb_sl.T)[None, :].astype(bf16)
        m["WkvbT"] = WkvbT_eff.astype(bf16)
        m["kconst"] = kconst.astype(bf16)
        m["WoutT"] = np.ascontiguousarray(
            Wout[:, hg * DPC:(hg + 1) * DPC].T).astype(bf16)
        in_maps.append(m)
    return in_maps


def kernel(**inputs):
    from concourse.bass_utils import run_bass_kernel_spmd
    if "nc" not in _CACHE:
        _CACHE["nc"] = _build(reps=1)
    nc = _CACHE["nc"]
    in_maps = _host_prep(**inputs)
    res = run_bass_kernel_spmd(nc, in_maps, core_ids=list(range(NCORES)))
    out = np.zeros((B, S, E), np.float32)
    for core in range(NCORES):
        out[core // HPC] += res.results[core]["out"].astype(np.float32)
    return out
